# revision 32
# baseline (speedup 1.0000x reference)
"""DGCNN forward kernel for Trainium2 (8 NeuronCores, data-parallel over batch).

Each core processes one point cloud (N=2048 points) end to end:
  4x EdgeConv (KNN k=20 + 1x1 conv + BN + LeakyReLU(0.2) + max over k)
  -> concat -> 1x1 conv to 1024 + BN + LeakyReLU -> global max+mean pool
  -> MLP 2048-512-256-128-2 with LeakyReLU(0.01).

Key algebraic rewrite: for monotone BN (scale>0) and LeakyReLU,
  max_k f(W @ [nbr - ctr, ctr]) = f(max_k(U[idx_k]) + V),
with U = Wl @ x, V = (Wr - Wl) @ x. This avoids materializing [N, K, 2C]
edge features; only U rows are gathered (dma_gather from a DRAM table).

Pipeline (per layer, per 128-point tile):
  PE:    S = x.x (fp32, exact ranking; + ones x (-0.5|x|^2) accumulate)
  ACT:   PSUM -> s_sb copies
  DVE:   top-24 via 3x(max8+find_index8) + 2x match_replace  (~18.3us)
  PE:    idx wrap via selr matmuls, batched per tile PAIR, deferred one
         pair so the PE queue never stalls behind the DVE topk
  GpSimd: gather split 4x640 idxs across SWDGE queues 0-3 (overlapped
         SDMA drains)
  DVE:   k-reduce max-tree (contiguous slices), deferred two pairs
  PE/ACT/DVE: y = lrelu(bn(V + max)) per point-quarter
fp16 is used only on value paths with slack (conv5 weights+activations,
U tables for the O>=128 layers); S ranking stays fp32 (quantized S flips
KNN boundary neighbors and fails the 2e-2 gate).
"""

import numpy as np
from contextlib import ExitStack

import concourse.bass as bass
import concourse.bacc as bacc
import concourse.tile as tile
from concourse import mybir
from concourse.bass_utils import run_bass_kernel_spmd
from concourse.masks import make_identity

F32 = mybir.dt.float32
FP16 = mybir.dt.float16
I16 = mybir.dt.int16
U32 = mybir.dt.uint32
AF = mybir.ActivationFunctionType
ALU = mybir.AluOpType
AX = mybir.AxisListType

B, N, KNN, P = 8, 2048, 20, 128
NT = N // P                      # 16 point tiles
NPAIR = NT // 2
EPS = 1e-5
NEG = -1e30
CONV = [(64, 3), (64, 64), (128, 64), (256, 128)]   # (O, C) of edge convs
LIN = [(512, 2048), (256, 512), (128, 256), (2, 128)]
LRELU_CONV = 0.2
LRELU_HEAD = 0.01
NQ = 4                           # SWDGE queues for gather
U_FP16 = [False, False, True, True]   # fp16 U table (needs O*2B % 256 == 0)


def _bn_fold(nc, sb, g_col, b_col, m_col, v_col, ncols, eps_col, pfx="bn"):
    """s = g * rsqrt(v + eps); t = b - m * s  (all [128, ncols] column tiles)."""
    s = sb.tile([P, ncols], F32, tag=f"{pfx}_s")
    t = sb.tile([P, ncols], F32, tag=f"{pfx}_t")
    tmp = sb.tile([P, ncols], F32, tag=f"{pfx}_tmp")
    nc.scalar.activation(out=tmp, in_=v_col, func=AF.Sqrt, bias=eps_col, scale=1.0)
    nc.vector.reciprocal(out=s, in_=tmp)
    nc.vector.tensor_mul(s, s, g_col)
    nc.vector.tensor_mul(tmp, m_col, s)
    nc.vector.tensor_sub(t, b_col, tmp)
    return s, t


def _emit(nc, tc, t_in, t_w, t_out, dbg):
    with ExitStack() as ctx:
        const = ctx.enter_context(tc.tile_pool(name="const", bufs=1))
        pers = ctx.enter_context(tc.tile_pool(name="pers", bufs=1))

        ident = const.tile([P, P], F32)
        make_identity(nc, ident[:])
        ones_col = const.tile([P, 1], F32)
        nc.vector.memset(ones_col, 1.0)
        ones_row = const.tile([1, P], F32)
        nc.vector.memset(ones_row, 1.0)
        eps_col = const.tile([P, 1], F32)
        nc.vector.memset(eps_col, EPS)
        # SELR[g][p, p'] = 1 iff p == g*16 + p' % 16  (wrapped-idx builder)
        selr = const.tile([P, 8, P], F32)
        for g in range(8):
            isrc = ident[:, g * 16:(g + 1) * 16]
            src_b = bass.AP(tensor=isrc.tensor, offset=isrc.offset,
                            ap=[isrc.ap[0], [0, 8], isrc.ap[1]])
            nc.vector.tensor_copy(
                out=selr[:, g, :].rearrange("p (o q) -> p o q", q=16), in_=src_b)

        # persistent feature maps, channels-first ([C(part), N(free)])
        x_in = pers.tile([3, N], F32, tag="x_in", name="x_in")
        x0 = pers.tile([64, N], F32, tag="x0", name="x0")
        x1 = pers.tile([64, N], F32, tag="x1", name="x1")
        x2 = pers.tile([P, N], F32, tag="x2", name="x2")
        x3 = pers.tile([P, 2 * N], F32, tag="x3", name="x3")  # 256 ch, 2 chunks
        x_aug = [x_in, x0, x1, x2]
        # fp16 shadows for conv5 chains
        xb = [
            pers.tile([3, N], FP16, tag="xh_in", name="xh_in"),
            pers.tile([64, N], FP16, tag="xb0", name="xb0"),
            pers.tile([64, N], FP16, tag="xb1", name="xb1"),
            pers.tile([P, N], FP16, tag="xb2", name="xb2"),
            pers.tile([P, 2 * N], FP16, tag="xb3", name="xb3"),
        ]
        # x_lo = x - fp32(x_hi): second fp16 term of the exact S split
        xl = [
            pers.tile([3, N], FP16, tag="xl_in", name="xl_in"),
            pers.tile([64, N], FP16, tag="xl0", name="xl0"),
            pers.tile([64, N], FP16, tag="xl1", name="xl1"),
            pers.tile([P, N], FP16, tag="xl2", name="xl2"),
        ]
        ident_h = const.tile([P, P], FP16)
        nc.vector.tensor_copy(out=ident_h, in_=ident)
        ones_row_h = const.tile([1, P], FP16)
        nc.vector.tensor_copy(out=ones_row_h, in_=ones_row)

        def transpose_to(ps_pool, tag, dst_ap, src_ap, rows_out, scale=1.0):
            """dst[f, p] = scale * src[p, f] via PE; src SBUF [p<=128, f<=128]."""
            pt = ps_pool.tile([P, P], F32, tag=tag)
            kdim = src_ap.shape[0]
            nc.tensor.transpose(out=pt[0:rows_out, 0:kdim], in_=src_ap,
                                identity=ident[0:kdim, 0:kdim])
            nc.scalar.activation(out=dst_ap, in_=pt[0:rows_out, 0:kdim],
                                 func=AF.Copy, scale=scale)

        # ---------------- input transpose: feat [N, 3] -> x_in rows 0..2 ------
        with tc.tile_pool(name="ps_setup", bufs=2, space="PSUM") as ps_setup, \
             tc.tile_pool(name="sb_setup", bufs=2) as sb_setup:
            for t in range(NT):
                ft = sb_setup.tile([P, 3], F32, tag="feat")
                nc.sync.dma_start(out=ft, in_=t_in["feat_xyz"][t * P:(t + 1) * P, :])
                transpose_to(ps_setup, "tr", x_in[0:3, t * P:(t + 1) * P], ft[:, :], 3)

        # ------------- weight prep for ALL layers (hoisted) -------------
        wlT_all, wvT_all, bn_all = [], [], []
        with tc.tile_pool(name="ps_wp", bufs=2, space="PSUM") as ps_wp, \
             tc.tile_pool(name="sb_wp", bufs=2) as sb_wp:
            for li, (O, C) in enumerate(CONV):
                OCH = (O + P - 1) // P
                wlT = pers.tile([P, O], F32, tag=f"wlT{li}")
                wvT = pers.tile([P, O], F32, tag=f"wvT{li}")
                for j in range(OCH):
                    ow = min(P, O - j * P)
                    wsb = sb_wp.tile([P, 2 * C], F32, tag="w_in")
                    nc.sync.dma_start(out=wsb[0:ow, :],
                                      in_=t_w[f"W{li}"][j * P:j * P + ow, :])
                    transpose_to(ps_wp, "wp", wlT[0:C, j * P:j * P + ow],
                                 wsb[0:ow, 0:C], C)
                    transpose_to(ps_wp, "wp", wvT[0:C, j * P:j * P + ow],
                                 wsb[0:ow, C:2 * C], C)
                nc.vector.tensor_sub(wvT[0:C, 0:O], wvT[0:C, 0:O], wlT[0:C, 0:O])
                g_col = sb_wp.tile([P, OCH], F32, tag=f"g{li}")
                b_col = sb_wp.tile([P, OCH], F32, tag=f"b{li}")
                m_col = sb_wp.tile([P, OCH], F32, tag=f"m{li}")
                v_col = sb_wp.tile([P, OCH], F32, tag=f"v{li}")
                if O < P:
                    for colt in (g_col, b_col, m_col, v_col):
                        nc.vector.memset(colt, 1.0)
                for j in range(OCH):
                    ow = min(P, O - j * P)
                    for colt, nm in ((g_col, "g"), (b_col, "b"), (m_col, "m"),
                                     (v_col, "v")):
                        nc.sync.dma_start(out=colt[0:ow, j:j + 1],
                                          in_=t_w[f"{nm}{li}"][j * P:j * P + ow, :])
                bn_all.append(_bn_fold(nc, pers, g_col, b_col, m_col, v_col,
                                       OCH, eps_col, pfx=f"bn{li}"))
                wlT_all.append(wlT)
                wvT_all.append(wvT)

        # =================== edge conv layers ===================
        for li, (O, C) in enumerate(CONV):
            OCH = (O + P - 1) // P  # o-chunks
            UDT = FP16 if U_FP16[li] else F32
            src = x_aug[li]
            dst = x_aug[li + 1] if li < 3 else x3
            with ExitStack() as lctx:
                sb = lctx.enter_context(tc.tile_pool(name=f"sb_l{li}", bufs=1))
                sbw = lctx.enter_context(tc.tile_pool(name=f"sbw_l{li}", bufs=2))
                sbg = lctx.enter_context(tc.tile_pool(name=f"sbg_l{li}", bufs=4))
                ps_s = lctx.enter_context(
                    tc.tile_pool(name=f"ps_s{li}", bufs=2, space="PSUM"))
                ps_w = lctx.enter_context(
                    tc.tile_pool(name=f"ps_w{li}", bufs=2, space="PSUM"))
                ps_y = lctx.enter_context(
                    tc.tile_pool(name=f"ps_y{li}", bufs=2, space="PSUM"))
                ps_m = lctx.enter_context(
                    tc.tile_pool(name=f"ps_m{li}", bufs=2, space="PSUM"))

                wlT, wvT = wlT_all[li], wvT_all[li]
                bn_s, bn_t = bn_all[li]

                # --- nsq row: -0.5 * sum_c x[c, m]^2 ([1, N]; PE rhs base
                # partition must be 0/32/64)
                nsq_t = sb.tile([1, N], F32, tag="nsq")
                xx = sb.tile([P, N], F32, tag="xx")
                nc.scalar.activation(out=xx[0:C, :], in_=src[0:C, 0:N], func=AF.Square)
                for q in range(4):
                    sl = slice(q * 512, (q + 1) * 512)
                    pq = ps_m.tile([1, 512], F32, tag="sm")
                    nc.tensor.matmul(out=pq, lhsT=ones_col[0:C, :], rhs=xx[0:C, sl],
                                     start=True, stop=True)
                    nc.scalar.activation(out=nsq_t[:, sl], in_=pq, func=AF.Copy,
                                         scale=-0.5)
                nsq_hi = sb.tile([1, N], FP16, tag="nsq_hi")
                nsq_lo = sb.tile([1, N], FP16, tag="nsq_lo")
                nc.scalar.activation(out=nsq_hi, in_=nsq_t, func=AF.Copy)
                nc.vector.tensor_tensor(out=nsq_lo, in0=nsq_t[:, :], in1=nsq_hi[:, :],
                                        op=ALU.subtract)
                if li == 0:
                    # x_in written by setup transposes: build its hi/lo shadows
                    nc.scalar.activation(out=xb[0][0:3, :], in_=src[0:3, :],
                                         func=AF.Copy)
                    nc.vector.tensor_tensor(out=xl[0][0:3, :], in0=src[0:3, :],
                                            in1=xb[0][0:3, :], op=ALU.subtract)
                xh, xlo = xb[li], xl[li]

                # --- U table -> DRAM (must land before the first gather;
                # emitted after pair 0's topks so PE starts with S(0))
                u_dram = t_w[f"Utab{li}"]

                def emit_U():
                    for t in range(NT):
                        pu = ps_m.tile([P, 512], F32, tag="sm")
                        nc.tensor.matmul(out=pu[:, 0:O],
                                         lhsT=src[0:C, t * P:(t + 1) * P],
                                         rhs=wlT[0:C, 0:O], start=True, stop=True)
                        usb = sbw.tile([P, O], UDT, tag="u_sb")
                        nc.scalar.activation(out=usb, in_=pu[:, 0:O], func=AF.Copy)
                        nc.sync.dma_start(out=u_dram[t * P:(t + 1) * P, :], in_=usb)

                m_all = sb.tile([P, NT * O], F32, tag="m_all")
                gt_tiles = [None] * NT

                def emit_S_topk(t):
                    s_sb = sbw.tile([P, N], F32, tag="s_sb")
                    for q in range(4):
                        sl = slice(q * 512, (q + 1) * 512)
                        pq = ps_s.tile([P, 512], F32, tag="s_ps")
                        tsl = slice(t * P, (t + 1) * P)
                        nc.tensor.matmul(out=pq, lhsT=xh[0:C, tsl],
                                         rhs=xh[0:C, sl], start=True, stop=False)
                        nc.tensor.matmul(out=pq, lhsT=xh[0:C, tsl],
                                         rhs=xlo[0:C, sl], start=False, stop=False)
                        nc.tensor.matmul(out=pq, lhsT=xlo[0:C, tsl],
                                         rhs=xh[0:C, sl], start=False, stop=False)
                        nc.tensor.matmul(out=pq, lhsT=xlo[0:C, tsl],
                                         rhs=xlo[0:C, sl], start=False, stop=False)
                        nc.tensor.matmul(out=pq, lhsT=ones_row_h, rhs=nsq_hi[:, sl],
                                         start=False, stop=False)
                        nc.tensor.matmul(out=pq, lhsT=ones_row_h, rhs=nsq_lo[:, sl],
                                         start=False, stop=True)
                        nc.scalar.activation(out=s_sb[:, sl], in_=pq, func=AF.Copy)
                    v24 = sbw.tile([P, 24], F32, tag="v24")
                    i24 = sbw.tile([P, 24], U32, tag="i24")
                    nc.vector.max(out=v24[:, 0:8], in_=s_sb)
                    nc.vector.max_index(out=i24[:, 0:8], in_max=v24[:, 0:8], in_values=s_sb)
                    nc.vector.match_replace(out=s_sb, in_to_replace=v24[:, 0:8],
                                            in_values=s_sb, imm_value=NEG)
                    nc.vector.max(out=v24[:, 8:16], in_=s_sb)
                    nc.vector.max_index(out=i24[:, 8:16], in_max=v24[:, 8:16], in_values=s_sb)
                    nc.vector.match_replace(out=s_sb, in_to_replace=v24[:, 8:16],
                                            in_values=s_sb, imm_value=NEG)
                    nc.vector.max(out=v24[:, 16:24], in_=s_sb)
                    nc.vector.max_index(out=i24[:, 16:24], in_max=v24[:, 16:24], in_values=s_sb)
                    return i24

                def emit_wrap_gather(t0, i24_pair):
                    """Wrap + gather for the tile pair (t0, t0+1): one idxf
                    [P, 2*KNN], 8 selr matmuls of 40 cols, 2x4 queue gathers."""
                    idxf = sbw.tile([P, 2, KNN], F32, tag="idxf")
                    nc.vector.tensor_copy(out=idxf[:, 0, :], in_=i24_pair[0][:, 0:KNN])
                    nc.vector.tensor_copy(out=idxf[:, 1, :], in_=i24_pair[1][:, 0:KNN])
                    pw = ps_w.tile([P, 2, KNN, 8], F32, tag="w_ps")
                    for g in range(8):
                        nc.tensor.matmul(
                            out=pw[:, :, :, g],
                            lhsT=selr[:, g, :],
                            rhs=idxf[:, :, :].rearrange("p t k -> p (t k)"),
                            start=True, stop=True, skip_group_check=True)
                    w16 = sbw.tile([P, 2 * 8 * KNN], I16, tag="w16")
                    nc.vector.tensor_copy(
                        out=w16, in_=pw[:, :, :, :].rearrange("p t k g -> p (t k g)"))
                    for dt_ in range(2):
                        gt = sbg.tile([P, KNN, O], UDT, tag="gather")
                        for q in range(NQ):
                            nc.gpsimd.dma_gather(
                                out_ap=gt[:, 5 * q:5 * (q + 1), :], in_ap=u_dram[:, :],
                                idxs_ap=w16[:, dt_ * 160 + 40 * q:dt_ * 160 + 40 * (q + 1)],
                                num_idxs=P * KNN // NQ, num_idxs_reg=P * KNN // NQ,
                                elem_size=O, single_packet=False, queue_num=q)
                        gt_tiles[t0 + dt_] = gt

                def emit_tree(t):
                    # m_all[:, t*O:(t+1)*O] = max over k of gt [P, 20, O]
                    # in-place tree on a single [P, 8*O] scratch
                    gt = gt_tiles[t]
                    gf = gt[:, :, :].rearrange("p k o -> p (k o)")
                    a = sbw.tile([P, 8 * O], UDT, tag="tr_a")
                    nc.vector.tensor_tensor(out=a, in0=gf[:, 0:8 * O],
                                            in1=gf[:, 8 * O:16 * O], op=ALU.max)
                    nc.vector.tensor_tensor(out=a[:, 0:4 * O], in0=a[:, 0:4 * O],
                                            in1=a[:, 4 * O:8 * O], op=ALU.max)
                    nc.vector.tensor_tensor(out=a[:, 4 * O:6 * O],
                                            in0=gf[:, 16 * O:18 * O],
                                            in1=gf[:, 18 * O:20 * O], op=ALU.max)
                    nc.vector.tensor_tensor(out=a[:, 0:2 * O], in0=a[:, 0:2 * O],
                                            in1=a[:, 2 * O:4 * O], op=ALU.max)
                    nc.vector.tensor_tensor(out=a[:, 0:2 * O], in0=a[:, 0:2 * O],
                                            in1=a[:, 4 * O:6 * O], op=ALU.max)
                    nc.vector.tensor_tensor(out=m_all[:, t * O:(t + 1) * O],
                                            in0=a[:, 0:O], in1=a[:, O:2 * O],
                                            op=ALU.max)

                def emit_y(qq):
                    """y for point quarter qq (tiles 4qq..4qq+3) into dst rows."""
                    for j in range(OCH):
                        ow = min(P, O - j * P)
                        py = ps_y.tile([P, 512], F32, tag="y_ps")
                        nc.tensor.matmul(out=py[0:ow, :],
                                         lhsT=wvT[0:C, j * P:j * P + ow],
                                         rhs=src[0:C, qq * 512:(qq + 1) * 512],
                                         start=True, stop=False,
                                         skip_group_check=True)
                        for tt in range(4):
                            t = qq * 4 + tt
                            msl = m_all[:, t * O + j * P: t * O + j * P + ow]
                            nc.tensor.matmul(
                                out=py[0:ow, tt * P:(tt + 1) * P],
                                lhsT=msl, rhs=ident,
                                is_transpose=True, start=False, stop=(tt == 3),
                                skip_group_check=True)
                        if li < 3:
                            fsl = slice(qq * 512, (qq + 1) * 512)
                            psl = slice(j * P, j * P + ow)
                            lsl = xl[li + 1][:, fsl][psl, :]
                        else:
                            fsl = slice(j * N + qq * 512, j * N + (qq + 1) * 512)
                            psl = slice(0, ow)
                            lsl = None
                        osl = dst[:, fsl][psl, :]
                        bsl = xb[li + 1][:, fsl][psl, :]
                        nc.scalar.activation(out=osl, in_=py[0:ow, :],
                                             func=AF.Identity, scale=bn_s[0:ow, j:j + 1],
                                             bias=bn_t[0:ow, j:j + 1])
                        nc.vector.scalar_tensor_tensor(
                            out=osl, in0=osl, scalar=LRELU_CONV, in1=osl,
                            op0=ALU.mult, op1=ALU.max)
                        # fp16 hi shadow (ACT) + lo residual (DVE)
                        nc.scalar.activation(out=bsl, in_=osl, func=AF.Copy)
                        if lsl is not None:
                            nc.vector.tensor_tensor(out=lsl, in0=osl, in1=bsl,
                                                    op=ALU.subtract)

                # ---- software-pipelined tile loop: wrap deferred one pair,
                # trees deferred two pairs (gather latency fully hidden) ----
                i24_tiles = [None] * NT
                for i in range(NPAIR + 1):
                    if i < NPAIR:
                        i24_tiles[2 * i] = emit_S_topk(2 * i)
                        if i >= 2:
                            emit_tree(2 * i - 4)
                        i24_tiles[2 * i + 1] = emit_S_topk(2 * i + 1)
                        if i == 0:
                            emit_U()
                        if i >= 2:
                            emit_tree(2 * i - 3)
                    if i >= 1:
                        emit_wrap_gather(2 * (i - 1), i24_tiles[2 * (i - 1):2 * i])
                    # tree(2i-3) completes quarter q when 2i-3 == 4q+3
                    if i >= 2 and (2 * i - 3) % 4 == 3:
                        emit_y((2 * i - 3) // 4)
                emit_tree(NT - 4)
                emit_tree(NT - 3)
                emit_tree(NT - 2)
                emit_tree(NT - 1)
                emit_y(3)
                if dbg:
                    if li < 3:
                        nc.sync.dma_start(out=t_out[f"dbg_x{li}"][:, :],
                                          in_=dst[0:O, :])
                    else:
                        nc.sync.dma_start(out=t_out[f"dbg_x{li}"][:, :], in_=dst[:, :])

        # =================== conv5 (1024) + pooling, fp16 ===================
        # cat chains: (fp16 tile, rows, W4 col offset, free offset in tile)
        chains = [
            (xb[1], 64, 0, 0),
            (xb[2], 64, 64, 0),
            (xb[3], 128, 128, 0),
            (xb[4], 128, 256, 0),
            (xb[4], 128, 384, N),
        ]
        p_cf = pers.tile([P, 16], F32, tag="p_cf")
        with ExitStack() as cctx:
            sb = cctx.enter_context(tc.tile_pool(name="sb_c5", bufs=1))
            sbw = cctx.enter_context(tc.tile_pool(name="sbw_c5", bufs=2))
            ps_h = cctx.enter_context(tc.tile_pool(name="ps_h", bufs=3, space="PSUM"))
            ps_sm = cctx.enter_context(tc.tile_pool(name="ps_smc", bufs=2, space="PSUM"))

            # W4T per chain: [C_chain, 8*128] fp16 tiles
            w4T = [sb.tile([P, 1024], FP16, tag=f"w4T_{ci}", name=f"w4T_{ci}")
                   for ci in range(5)]
            for j in range(8):
                wsb = sbw.tile([P, 512], F32, tag="w4_in")
                nc.sync.dma_start(out=wsb, in_=t_w["W4"][j * P:(j + 1) * P, :])
                for ci, (xt, crow, c0, fo) in enumerate(chains):
                    transpose_to(ps_sm, "sm", w4T[ci][0:crow, j * P:(j + 1) * P],
                                 wsb[:, c0:c0 + crow], crow)

            g4 = sb.tile([P, 8], F32, tag="g4")
            b4 = sb.tile([P, 8], F32, tag="b4")
            m4 = sb.tile([P, 8], F32, tag="m4")
            v4 = sb.tile([P, 8], F32, tag="v4")
            for j in range(8):
                for colt, nm in ((g4, "g"), (b4, "b"), (m4, "m"), (v4, "v")):
                    nc.sync.dma_start(out=colt[:, j:j + 1],
                                      in_=t_w[f"{nm}4"][j * P:(j + 1) * P, :])
            s4, t4 = _bn_fold(nc, sb, g4, b4, m4, v4, 8, eps_col)

            for j in range(8):
                h_sb = sbw.tile([P, N], F32, tag="h_sb")
                mean_part = sbw.tile([P, 4], F32, tag="mean_part")
                for q in range(4):
                    ph = ps_h.tile([P, 512], F32, tag="h_ps")
                    for ci, (xt, crow, c0, fo) in enumerate(chains):
                        nc.tensor.matmul(out=ph,
                                         lhsT=w4T[ci][0:crow, j * P:(j + 1) * P],
                                         rhs=xt[0:crow, fo + q * 512: fo + (q + 1) * 512],
                                         start=(ci == 0), stop=(ci == 4))
                    sl = slice(q * 512, (q + 1) * 512)
                    nc.scalar.activation(out=h_sb[:, sl], in_=ph, func=AF.Identity,
                                         scale=s4[:, j:j + 1], bias=t4[:, j:j + 1])
                    nc.vector.scalar_tensor_tensor(
                        out=h_sb[:, sl], in0=h_sb[:, sl], scalar=LRELU_CONV,
                        in1=h_sb[:, sl], op0=ALU.mult, op1=ALU.max,
                        accum_out=mean_part[:, q:q + 1])
                # pools
                nc.vector.tensor_reduce(out=p_cf[:, j:j + 1], in_=h_sb[:, :],
                                        axis=AX.X, op=ALU.max)
                nc.vector.tensor_reduce(out=p_cf[:, 8 + j:9 + j], in_=mean_part[:, :],
                                        axis=AX.X, op=ALU.add)
            nc.vector.tensor_scalar_mul(p_cf[:, 8:16], p_cf[:, 8:16], 1.0 / N)
            if dbg:
                nc.sync.dma_start(out=t_out["dbg_p"][:, :], in_=p_cf[:, :])

        # =================== MLP head (broadcast + DVE dot-products) ==========
        with ExitStack() as hctx:
            sb = hctx.enter_context(tc.tile_pool(name="sb_head", bufs=1))
            sbw = hctx.enter_context(tc.tile_pool(name="sbw_head", bufs=2))
            ps_hd = hctx.enter_context(tc.tile_pool(name="ps_hd", bufs=2, space="PSUM"))

            def lin(name, src_col, incols, w_dram, out_dim, alpha):
                """dst [128, ceil(out/128)] = lrelu(alpha)(W @ src).
                src_col [128, incols] column tile (in_dim = 128*incols)."""
                in_dim = P * incols
                och = (out_dim + P - 1) // P
                orows = min(P, out_dim)
                # broadcast src over partitions: bcast[p', c] = src[c]
                bcast = sb.tile([P, in_dim], F32, tag=f"{name}_bc")
                for j in range(incols):
                    pT = ps_hd.tile([1, P], F32, tag="hd_tr")
                    nc.tensor.transpose(out=pT, in_=src_col[:, j:j + 1],
                                        identity=ident)
                    rowj = sbw.tile([1, P], F32, tag="hd_row")
                    nc.scalar.activation(out=rowj, in_=pT, func=AF.Copy)
                    pb = ps_hd.tile([P, P], F32, tag="hd_bc")
                    nc.tensor.matmul(out=pb, lhsT=ones_row, rhs=rowj,
                                     start=True, stop=True)
                    nc.scalar.activation(out=bcast[:, j * P:(j + 1) * P], in_=pb,
                                         func=AF.Copy)
                dst = sb.tile([P, och], F32, tag=f"{name}_out")
                for ot in range(och):
                    orw = min(P, out_dim - ot * P)
                    wsb = sbw.tile([P, in_dim], F32, tag=f"{name}_w")
                    nc.sync.dma_start(out=wsb[0:orw, :],
                                      in_=w_dram[ot * P:ot * P + orw, :])
                    prod = sbw.tile([P, in_dim], F32, tag=f"{name}_prod")
                    nc.vector.tensor_mul(prod[0:orw, :], wsb[0:orw, :], bcast[0:orw, :])
                    nc.vector.tensor_reduce(out=dst[0:orw, ot:ot + 1],
                                            in_=prod[0:orw, :], axis=AX.X, op=ALU.add)
                if alpha is not None:
                    nc.vector.scalar_tensor_tensor(
                        out=dst[0:orows, :], in0=dst[0:orows, :], scalar=alpha,
                        in1=dst[0:orows, :], op0=ALU.mult, op1=ALU.max)
                return dst

            y1 = lin("y1", p_cf, 16, t_w["L1"], 512, LRELU_HEAD)
            y2 = lin("y2", y1, 4, t_w["L2"], 256, LRELU_HEAD)
            y3 = lin("y3", y2, 2, t_w["L3"], 128, LRELU_HEAD)
            y4 = lin("y4", y3, 1, t_w["L4"], 2, None)
            osb = sb.tile([2, 1], F32, tag="out_sb")
            nc.vector.tensor_copy(out=osb, in_=y4[0:2, 0:1])
            nc.sync.dma_start(out=t_out["out"][:, :], in_=osb)


_PROG_CACHE = {}


def _build(dbg=False):
    key = ("v3", dbg)
    if key in _PROG_CACHE:
        return _PROG_CACHE[key]
    nc = bacc.Bacc("TRN2", target_bir_lowering=False, debug=False, num_devices=B,
                   num_swdge_queues=NQ)
    t_in = {"feat_xyz": nc.declare_dram_parameter("feat_xyz", [N, 3], F32, isOutput=False)}
    t_w = {}
    for li, (O, C) in enumerate(CONV + [(1024, 512)]):
        wshape = [O, 2 * C] if li < 4 else [O, C]
        t_w[f"W{li}"] = nc.declare_dram_parameter(f"W{li}", wshape, F32, isOutput=False)
        for nm in "gbmv":
            t_w[f"{nm}{li}"] = nc.declare_dram_parameter(f"{nm}{li}", [O, 1], F32,
                                                         isOutput=False)
    for j, (o, c) in enumerate(LIN):
        t_w[f"L{j+1}"] = nc.declare_dram_parameter(f"L{j+1}", [o, c], F32, isOutput=False)
    for li, (O, C) in enumerate(CONV):
        t_w[f"Utab{li}"] = nc.dram_tensor(f"Utab{li}", [N, O],
                                          FP16 if U_FP16[li] else F32)
    t_out = {"out": nc.declare_dram_parameter("out", [2, 1], F32, isOutput=True)}
    if dbg:
        for li, (O, C) in enumerate(CONV):
            sh = [P, 2 * N] if O == 256 else [O, N]
            t_out[f"dbg_x{li}"] = nc.declare_dram_parameter(f"dbg_x{li}", sh, F32,
                                                            isOutput=True)
        t_out["dbg_p"] = nc.declare_dram_parameter("dbg_p", [P, 16], F32, isOutput=True)

    with tile.TileContext(nc) as tc:
        _emit(nc, tc, t_in, t_w, t_out, dbg)
    nc.compile()
    _PROG_CACHE[key] = nc
    return nc


def _make_in_maps(inputs):
    feat = np.ascontiguousarray(np.asarray(inputs["feat_xyz"], dtype=np.float32))
    common = {}
    for li in range(5):
        common[f"W{li}"] = np.ascontiguousarray(np.asarray(inputs[f"W{li}"], np.float32))
        for nm in "gbmv":
            common[f"{nm}{li}"] = np.ascontiguousarray(
                np.asarray(inputs[f"{nm}{li}"], np.float32).reshape(-1, 1))
    for j in range(1, 5):
        common[f"L{j}"] = np.ascontiguousarray(np.asarray(inputs[f"L{j}"], np.float32))
    return [dict(common, feat_xyz=np.ascontiguousarray(feat[b])) for b in range(B)]


def run(inputs, dbg=False, trace=False, **kw):
    nc = _build(dbg)
    in_maps = _make_in_maps(inputs)
    return run_bass_kernel_spmd(nc, in_maps, list(range(B)), trace=trace, **kw)


def kernel(**inputs):
    res = run(inputs).results
    out = np.stack([res[b]["out"][:, 0] for b in range(B)], axis=0)
    return out.astype(np.float32)


# revision 33
# speedup vs baseline: 1.2208x; 1.2208x over previous
"""DGCNN forward kernel for Trainium2 (8 NeuronCores, data-parallel over batch).

Each core processes one point cloud (N=2048 points) end to end:
  4x EdgeConv (KNN k=20 + 1x1 conv + BN + LeakyReLU(0.2) + max over k)
  -> concat -> 1x1 conv to 1024 + BN + LeakyReLU -> global max+mean pool
  -> MLP 2048-512-256-128-2 with LeakyReLU(0.01).

Key algebraic rewrite: for monotone BN (scale>0) and LeakyReLU,
  max_k f(W @ [nbr - ctr, ctr]) = f(max_k(U[idx_k]) + V),
with U = Wl @ x, V = (Wr - Wl) @ x. This avoids materializing [N, K, 2C]
edge features; only U rows are gathered (dma_gather from a DRAM table).

Pipeline (per layer, per 128-point tile):
  PE:    S = x.x (fp32, exact ranking; + ones x (-0.5|x|^2) accumulate)
  ACT:   PSUM -> s_sb copies
  DVE:   top-24 via 3x(max8+find_index8) + 2x match_replace  (~18.3us)
  PE:    idx wrap via selr matmuls, batched per tile PAIR, deferred one
         pair so the PE queue never stalls behind the DVE topk
  GpSimd: gather split 4x640 idxs across SWDGE queues 0-3 (overlapped
         SDMA drains)
  DVE:   k-reduce max-tree (contiguous slices), deferred two pairs
  PE/ACT/DVE: y = lrelu(bn(V + max)) per point-quarter
fp16 is used only on value paths with slack (conv5 weights+activations,
U tables for the O>=128 layers); S ranking stays fp32 (quantized S flips
KNN boundary neighbors and fails the 2e-2 gate).
"""

import numpy as np
from contextlib import ExitStack

import concourse.bass as bass
import concourse.bacc as bacc
import concourse.tile as tile
from concourse import mybir
from concourse.bass_utils import run_bass_kernel_spmd
from concourse.masks import make_identity

F32 = mybir.dt.float32
FP16 = mybir.dt.float16
I16 = mybir.dt.int16
U32 = mybir.dt.uint32
AF = mybir.ActivationFunctionType
ALU = mybir.AluOpType
AX = mybir.AxisListType

B, N, KNN, P = 8, 2048, 20, 128
NT = N // P                      # 16 point tiles
NPAIR = NT // 2
EPS = 1e-5
NEG = -1e30
CONV = [(64, 3), (64, 64), (128, 64), (256, 128)]   # (O, C) of edge convs
LIN = [(512, 2048), (256, 512), (128, 256), (2, 128)]
LRELU_CONV = 0.2
LRELU_HEAD = 0.01
NQ = 4                           # SWDGE queues for gather
U_FP16 = [False, False, True, True]   # fp16 U table (needs O*2B % 256 == 0)


def _bn_fold(nc, sb, g_col, b_col, m_col, v_col, ncols, eps_col, pfx="bn"):
    """s = g * rsqrt(v + eps); t = b - m * s  (all [128, ncols] column tiles)."""
    s = sb.tile([P, ncols], F32, tag=f"{pfx}_s")
    t = sb.tile([P, ncols], F32, tag=f"{pfx}_t")
    tmp = sb.tile([P, ncols], F32, tag=f"{pfx}_tmp")
    nc.scalar.activation(out=tmp, in_=v_col, func=AF.Sqrt, bias=eps_col, scale=1.0)
    nc.vector.reciprocal(out=s, in_=tmp)
    nc.vector.tensor_mul(s, s, g_col)
    nc.vector.tensor_mul(tmp, m_col, s)
    nc.vector.tensor_sub(t, b_col, tmp)
    return s, t


def _emit(nc, tc, t_in, t_w, t_out, dbg):
    with ExitStack() as ctx:
        const = ctx.enter_context(tc.tile_pool(name="const", bufs=1))
        pers = ctx.enter_context(tc.tile_pool(name="pers", bufs=1))

        ident = const.tile([P, P], F32)
        make_identity(nc, ident[:])
        ones_col = const.tile([P, 1], F32)
        nc.vector.memset(ones_col, 1.0)
        ones_row = const.tile([1, P], F32)
        nc.vector.memset(ones_row, 1.0)
        eps_col = const.tile([P, 1], F32)
        nc.vector.memset(eps_col, EPS)
        # SELR[g][p, p'] = 1 iff p == g*16 + p' % 16  (wrapped-idx builder)
        selr = const.tile([P, 8, P], F32)
        for g in range(8):
            isrc = ident[:, g * 16:(g + 1) * 16]
            src_b = bass.AP(tensor=isrc.tensor, offset=isrc.offset,
                            ap=[isrc.ap[0], [0, 8], isrc.ap[1]])
            nc.vector.tensor_copy(
                out=selr[:, g, :].rearrange("p (o q) -> p o q", q=16), in_=src_b)

        # persistent feature maps, channels-first ([C(part), N(free)])
        x_in = pers.tile([3, N], F32, tag="x_in", name="x_in")
        x0 = pers.tile([64, N], F32, tag="x0", name="x0")
        x1 = pers.tile([64, N], F32, tag="x1", name="x1")
        x2 = pers.tile([P, N], F32, tag="x2", name="x2")
        x3 = pers.tile([P, 2 * N], F32, tag="x3", name="x3")  # 256 ch, 2 chunks
        x_aug = [x_in, x0, x1, x2]
        # fp16 shadows for conv5 chains
        xb = [
            pers.tile([64, N], FP16, tag="xb0", name="xb0"),
            pers.tile([64, N], FP16, tag="xb1", name="xb1"),
            pers.tile([P, N], FP16, tag="xb2", name="xb2"),
            pers.tile([P, 2 * N], FP16, tag="xb3", name="xb3"),
        ]

        def transpose_to(ps_pool, tag, dst_ap, src_ap, rows_out, scale=1.0):
            """dst[f, p] = scale * src[p, f] via PE; src SBUF [p<=128, f<=128]."""
            pt = ps_pool.tile([P, P], F32, tag=tag)
            kdim = src_ap.shape[0]
            nc.tensor.transpose(out=pt[0:rows_out, 0:kdim], in_=src_ap,
                                identity=ident[0:kdim, 0:kdim])
            nc.scalar.activation(out=dst_ap, in_=pt[0:rows_out, 0:kdim],
                                 func=AF.Copy, scale=scale)

        # ---------------- input transpose: feat [N, 3] -> x_in rows 0..2 ------
        with tc.tile_pool(name="ps_setup", bufs=2, space="PSUM") as ps_setup, \
             tc.tile_pool(name="sb_setup", bufs=2) as sb_setup:
            for t in range(NT):
                ft = sb_setup.tile([P, 3], F32, tag="feat")
                nc.sync.dma_start(out=ft, in_=t_in["feat_xyz"][t * P:(t + 1) * P, :])
                transpose_to(ps_setup, "tr", x_in[0:3, t * P:(t + 1) * P], ft[:, :], 3)

        # ------------- weight prep for ALL layers (hoisted) -------------
        wlT_all, wvT_all, bn_all = [], [], []
        with tc.tile_pool(name="ps_wp", bufs=2, space="PSUM") as ps_wp, \
             tc.tile_pool(name="sb_wp", bufs=2) as sb_wp:
            for li, (O, C) in enumerate(CONV):
                OCH = (O + P - 1) // P
                wlT = pers.tile([P, O], F32, tag=f"wlT{li}")
                wvT = pers.tile([P, O], F32, tag=f"wvT{li}")
                for j in range(OCH):
                    ow = min(P, O - j * P)
                    wsb = sb_wp.tile([P, 2 * C], F32, tag="w_in")
                    nc.sync.dma_start(out=wsb[0:ow, :],
                                      in_=t_w[f"W{li}"][j * P:j * P + ow, :])
                    transpose_to(ps_wp, "wp", wlT[0:C, j * P:j * P + ow],
                                 wsb[0:ow, 0:C], C)
                    transpose_to(ps_wp, "wp", wvT[0:C, j * P:j * P + ow],
                                 wsb[0:ow, C:2 * C], C)
                nc.vector.tensor_sub(wvT[0:C, 0:O], wvT[0:C, 0:O], wlT[0:C, 0:O])
                g_col = sb_wp.tile([P, OCH], F32, tag=f"g{li}")
                b_col = sb_wp.tile([P, OCH], F32, tag=f"b{li}")
                m_col = sb_wp.tile([P, OCH], F32, tag=f"m{li}")
                v_col = sb_wp.tile([P, OCH], F32, tag=f"v{li}")
                if O < P:
                    for colt in (g_col, b_col, m_col, v_col):
                        nc.vector.memset(colt, 1.0)
                for j in range(OCH):
                    ow = min(P, O - j * P)
                    for colt, nm in ((g_col, "g"), (b_col, "b"), (m_col, "m"),
                                     (v_col, "v")):
                        nc.sync.dma_start(out=colt[0:ow, j:j + 1],
                                          in_=t_w[f"{nm}{li}"][j * P:j * P + ow, :])
                bn_all.append(_bn_fold(nc, pers, g_col, b_col, m_col, v_col,
                                       OCH, eps_col, pfx=f"bn{li}"))
                wlT_all.append(wlT)
                wvT_all.append(wvT)

        # =================== edge conv layers ===================
        for li, (O, C) in enumerate(CONV):
            OCH = (O + P - 1) // P  # o-chunks
            UDT = FP16 if U_FP16[li] else F32
            src = x_aug[li]
            dst = x_aug[li + 1] if li < 3 else x3
            with ExitStack() as lctx:
                sb = lctx.enter_context(tc.tile_pool(name=f"sb_l{li}", bufs=1))
                sbw = lctx.enter_context(tc.tile_pool(name=f"sbw_l{li}", bufs=2))
                sbg = lctx.enter_context(tc.tile_pool(name=f"sbg_l{li}", bufs=4))
                ps_s = lctx.enter_context(
                    tc.tile_pool(name=f"ps_s{li}", bufs=2, space="PSUM"))
                ps_w = lctx.enter_context(
                    tc.tile_pool(name=f"ps_w{li}", bufs=2, space="PSUM"))
                ps_y = lctx.enter_context(
                    tc.tile_pool(name=f"ps_y{li}", bufs=2, space="PSUM"))
                ps_m = lctx.enter_context(
                    tc.tile_pool(name=f"ps_m{li}", bufs=2, space="PSUM"))

                wlT, wvT = wlT_all[li], wvT_all[li]
                bn_s, bn_t = bn_all[li]

                # --- nsq row: -0.5 * sum_c x[c, m]^2 ([1, N]; PE rhs base
                # partition must be 0/32/64)
                nsq_t = sb.tile([1, N], F32, tag="nsq")
                xx = sb.tile([P, N], F32, tag="xx")
                nc.scalar.activation(out=xx[0:C, :], in_=src[0:C, 0:N], func=AF.Square)
                for q in range(4):
                    sl = slice(q * 512, (q + 1) * 512)
                    pq = ps_m.tile([1, 512], F32, tag="sm")
                    nc.tensor.matmul(out=pq, lhsT=ones_col[0:C, :], rhs=xx[0:C, sl],
                                     start=True, stop=True)
                    nc.scalar.activation(out=nsq_t[:, sl], in_=pq, func=AF.Copy,
                                         scale=-0.5)

                # --- U table -> DRAM (all 16 tiles must land before gathers)
                u_dram = t_w[f"Utab{li}"]
                for t in range(NT):
                    pu = ps_m.tile([P, 512], F32, tag="sm")
                    nc.tensor.matmul(out=pu[:, 0:O], lhsT=src[0:C, t * P:(t + 1) * P],
                                     rhs=wlT[0:C, 0:O], start=True, stop=True)
                    usb = sbw.tile([P, O], UDT, tag="u_sb")
                    nc.scalar.activation(out=usb, in_=pu[:, 0:O], func=AF.Copy)
                    nc.sync.dma_start(out=u_dram[t * P:(t + 1) * P, :], in_=usb)

                m_all = sb.tile([P, NT * O], F32, tag="m_all")
                gt_tiles = [None] * NT

                def emit_S_topk(t):
                    s_sb = sbw.tile([P, N], F32, tag="s_sb")
                    for q in range(4):
                        sl = slice(q * 512, (q + 1) * 512)
                        pq = ps_s.tile([P, 512], F32, tag="s_ps")
                        nc.tensor.matmul(out=pq, lhsT=src[0:C, t * P:(t + 1) * P],
                                         rhs=src[0:C, sl],
                                         start=True, stop=False)
                        nc.tensor.matmul(out=pq, lhsT=ones_row, rhs=nsq_t[:, sl],
                                         start=False, stop=True)
                        nc.scalar.activation(out=s_sb[:, sl], in_=pq, func=AF.Copy)
                    v24 = sbw.tile([P, 24], F32, tag="v24")
                    i24 = sbw.tile([P, 24], U32, tag="i24")
                    nc.vector.max(out=v24[:, 0:8], in_=s_sb)
                    nc.vector.max_index(out=i24[:, 0:8], in_max=v24[:, 0:8], in_values=s_sb)
                    nc.vector.match_replace(out=s_sb, in_to_replace=v24[:, 0:8],
                                            in_values=s_sb, imm_value=NEG)
                    nc.vector.max(out=v24[:, 8:16], in_=s_sb)
                    nc.vector.max_index(out=i24[:, 8:16], in_max=v24[:, 8:16], in_values=s_sb)
                    nc.vector.match_replace(out=s_sb, in_to_replace=v24[:, 8:16],
                                            in_values=s_sb, imm_value=NEG)
                    nc.vector.max(out=v24[:, 16:24], in_=s_sb)
                    nc.vector.max_index(out=i24[:, 16:24], in_max=v24[:, 16:24], in_values=s_sb)
                    return i24

                def emit_wrap_gather(t0, i24_pair):
                    """Wrap + gather for the tile pair (t0, t0+1): one idxf
                    [P, 2*KNN], 8 selr matmuls of 40 cols, 2x4 queue gathers."""
                    idxf = sbw.tile([P, 2, KNN], F32, tag="idxf")
                    nc.vector.tensor_copy(out=idxf[:, 0, :], in_=i24_pair[0][:, 0:KNN])
                    nc.vector.tensor_copy(out=idxf[:, 1, :], in_=i24_pair[1][:, 0:KNN])
                    pw = ps_w.tile([P, 2, KNN, 8], F32, tag="w_ps")
                    for g in range(8):
                        nc.tensor.matmul(
                            out=pw[:, :, :, g],
                            lhsT=selr[:, g, :],
                            rhs=idxf[:, :, :].rearrange("p t k -> p (t k)"),
                            start=True, stop=True, skip_group_check=True)
                    w16 = sbw.tile([P, 2 * 8 * KNN], I16, tag="w16")
                    nc.vector.tensor_copy(
                        out=w16, in_=pw[:, :, :, :].rearrange("p t k g -> p (t k g)"))
                    for dt_ in range(2):
                        gt = sbg.tile([P, KNN, O], UDT, tag="gather")
                        for q in range(NQ):
                            nc.gpsimd.dma_gather(
                                out_ap=gt[:, 5 * q:5 * (q + 1), :], in_ap=u_dram[:, :],
                                idxs_ap=w16[:, dt_ * 160 + 40 * q:dt_ * 160 + 40 * (q + 1)],
                                num_idxs=P * KNN // NQ, num_idxs_reg=P * KNN // NQ,
                                elem_size=O, single_packet=False, queue_num=q)
                        gt_tiles[t0 + dt_] = gt

                def emit_tree(t):
                    # m_all[:, t*O:(t+1)*O] = max over k of gt [P, 20, O]
                    # in-place tree on a single [P, 8*O] scratch
                    gt = gt_tiles[t]
                    gf = gt[:, :, :].rearrange("p k o -> p (k o)")
                    a = sbw.tile([P, 8 * O], UDT, tag="tr_a")
                    nc.vector.tensor_tensor(out=a, in0=gf[:, 0:8 * O],
                                            in1=gf[:, 8 * O:16 * O], op=ALU.max)
                    nc.vector.tensor_tensor(out=a[:, 0:4 * O], in0=a[:, 0:4 * O],
                                            in1=a[:, 4 * O:8 * O], op=ALU.max)
                    nc.vector.tensor_tensor(out=a[:, 4 * O:6 * O],
                                            in0=gf[:, 16 * O:18 * O],
                                            in1=gf[:, 18 * O:20 * O], op=ALU.max)
                    nc.vector.tensor_tensor(out=a[:, 0:2 * O], in0=a[:, 0:2 * O],
                                            in1=a[:, 2 * O:4 * O], op=ALU.max)
                    nc.vector.tensor_tensor(out=a[:, 0:2 * O], in0=a[:, 0:2 * O],
                                            in1=a[:, 4 * O:6 * O], op=ALU.max)
                    nc.vector.tensor_tensor(out=m_all[:, t * O:(t + 1) * O],
                                            in0=a[:, 0:O], in1=a[:, O:2 * O],
                                            op=ALU.max)

                def emit_y(qq):
                    """y for point quarter qq (tiles 4qq..4qq+3) into dst rows."""
                    for j in range(OCH):
                        ow = min(P, O - j * P)
                        py = ps_y.tile([P, 512], F32, tag="y_ps")
                        nc.tensor.matmul(out=py[0:ow, :],
                                         lhsT=wvT[0:C, j * P:j * P + ow],
                                         rhs=src[0:C, qq * 512:(qq + 1) * 512],
                                         start=True, stop=False,
                                         skip_group_check=True)
                        for tt in range(4):
                            t = qq * 4 + tt
                            msl = m_all[:, t * O + j * P: t * O + j * P + ow]
                            nc.tensor.matmul(
                                out=py[0:ow, tt * P:(tt + 1) * P],
                                lhsT=msl, rhs=ident,
                                is_transpose=True, start=False, stop=(tt == 3),
                                skip_group_check=True)
                        if li < 3:
                            fsl = slice(qq * 512, (qq + 1) * 512)
                            psl = slice(j * P, j * P + ow)
                        else:
                            fsl = slice(j * N + qq * 512, j * N + (qq + 1) * 512)
                            psl = slice(0, ow)
                        osl = dst[:, fsl][psl, :]
                        bsl = xb[li][:, fsl][psl, :]
                        nc.scalar.activation(out=osl, in_=py[0:ow, :],
                                             func=AF.Identity, scale=bn_s[0:ow, j:j + 1],
                                             bias=bn_t[0:ow, j:j + 1])
                        nc.vector.scalar_tensor_tensor(
                            out=osl, in0=osl, scalar=LRELU_CONV, in1=osl,
                            op0=ALU.mult, op1=ALU.max)
                        # fp16 shadow for conv5 (ACT; keeps DVE free)
                        nc.scalar.activation(out=bsl, in_=osl, func=AF.Copy)

                # ---- software-pipelined tile loop: wrap deferred one pair,
                # trees deferred two pairs (gather latency fully hidden) ----
                i24_tiles = [None] * NT
                for i in range(NPAIR + 1):
                    if i < NPAIR:
                        i24_tiles[2 * i] = emit_S_topk(2 * i)
                        if i >= 2:
                            emit_tree(2 * i - 4)
                        i24_tiles[2 * i + 1] = emit_S_topk(2 * i + 1)
                        if i >= 2:
                            emit_tree(2 * i - 3)
                    if i >= 1:
                        emit_wrap_gather(2 * (i - 1), i24_tiles[2 * (i - 1):2 * i])
                    # tree(2i-3) completes quarter q when 2i-3 == 4q+3
                    if i >= 2 and (2 * i - 3) % 4 == 3:
                        emit_y((2 * i - 3) // 4)
                emit_tree(NT - 4)
                emit_tree(NT - 3)
                emit_tree(NT - 2)
                emit_tree(NT - 1)
                emit_y(3)
                if dbg:
                    if li < 3:
                        nc.sync.dma_start(out=t_out[f"dbg_x{li}"][:, :],
                                          in_=dst[0:O, :])
                    else:
                        nc.sync.dma_start(out=t_out[f"dbg_x{li}"][:, :], in_=dst[:, :])

        # =================== conv5 (1024) + pooling, fp16 ===================
        # cat chains: (fp16 tile, rows, W4 col offset, free offset in tile)
        chains = [
            (xb[0], 64, 0, 0),
            (xb[1], 64, 64, 0),
            (xb[2], 128, 128, 0),
            (xb[3], 128, 256, 0),
            (xb[3], 128, 384, N),
        ]
        p_cf = pers.tile([P, 16], F32, tag="p_cf")
        with ExitStack() as cctx:
            sb = cctx.enter_context(tc.tile_pool(name="sb_c5", bufs=1))
            sbw = cctx.enter_context(tc.tile_pool(name="sbw_c5", bufs=2))
            ps_h = cctx.enter_context(tc.tile_pool(name="ps_h", bufs=3, space="PSUM"))
            ps_sm = cctx.enter_context(tc.tile_pool(name="ps_smc", bufs=2, space="PSUM"))

            # W4T per chain: [C_chain, 8*128] fp16 tiles
            w4T = [sb.tile([P, 1024], FP16, tag=f"w4T_{ci}", name=f"w4T_{ci}")
                   for ci in range(5)]
            for j in range(8):
                wsb = sbw.tile([P, 512], F32, tag="w4_in")
                nc.sync.dma_start(out=wsb, in_=t_w["W4"][j * P:(j + 1) * P, :])
                for ci, (xt, crow, c0, fo) in enumerate(chains):
                    transpose_to(ps_sm, "sm", w4T[ci][0:crow, j * P:(j + 1) * P],
                                 wsb[:, c0:c0 + crow], crow)

            g4 = sb.tile([P, 8], F32, tag="g4")
            b4 = sb.tile([P, 8], F32, tag="b4")
            m4 = sb.tile([P, 8], F32, tag="m4")
            v4 = sb.tile([P, 8], F32, tag="v4")
            for j in range(8):
                for colt, nm in ((g4, "g"), (b4, "b"), (m4, "m"), (v4, "v")):
                    nc.sync.dma_start(out=colt[:, j:j + 1],
                                      in_=t_w[f"{nm}4"][j * P:(j + 1) * P, :])
            s4, t4 = _bn_fold(nc, sb, g4, b4, m4, v4, 8, eps_col)

            for j in range(8):
                h_sb = sbw.tile([P, N], F32, tag="h_sb")
                mean_part = sbw.tile([P, 4], F32, tag="mean_part")
                for q in range(4):
                    ph = ps_h.tile([P, 512], F32, tag="h_ps")
                    for ci, (xt, crow, c0, fo) in enumerate(chains):
                        nc.tensor.matmul(out=ph,
                                         lhsT=w4T[ci][0:crow, j * P:(j + 1) * P],
                                         rhs=xt[0:crow, fo + q * 512: fo + (q + 1) * 512],
                                         start=(ci == 0), stop=(ci == 4))
                    sl = slice(q * 512, (q + 1) * 512)
                    nc.scalar.activation(out=h_sb[:, sl], in_=ph, func=AF.Identity,
                                         scale=s4[:, j:j + 1], bias=t4[:, j:j + 1])
                    nc.vector.scalar_tensor_tensor(
                        out=h_sb[:, sl], in0=h_sb[:, sl], scalar=LRELU_CONV,
                        in1=h_sb[:, sl], op0=ALU.mult, op1=ALU.max,
                        accum_out=mean_part[:, q:q + 1])
                # pools
                nc.vector.tensor_reduce(out=p_cf[:, j:j + 1], in_=h_sb[:, :],
                                        axis=AX.X, op=ALU.max)
                nc.vector.tensor_reduce(out=p_cf[:, 8 + j:9 + j], in_=mean_part[:, :],
                                        axis=AX.X, op=ALU.add)
            nc.vector.tensor_scalar_mul(p_cf[:, 8:16], p_cf[:, 8:16], 1.0 / N)
            if dbg:
                nc.sync.dma_start(out=t_out["dbg_p"][:, :], in_=p_cf[:, :])

        # =================== MLP head (broadcast + DVE dot-products) ==========
        with ExitStack() as hctx:
            sb = hctx.enter_context(tc.tile_pool(name="sb_head", bufs=1))
            sbw = hctx.enter_context(tc.tile_pool(name="sbw_head", bufs=2))
            ps_hd = hctx.enter_context(tc.tile_pool(name="ps_hd", bufs=2, space="PSUM"))

            def lin(name, src_col, incols, w_dram, out_dim, alpha):
                """dst [128, ceil(out/128)] = lrelu(alpha)(W @ src).
                src_col [128, incols] column tile (in_dim = 128*incols)."""
                in_dim = P * incols
                och = (out_dim + P - 1) // P
                orows = min(P, out_dim)
                # broadcast src over partitions: bcast[p', c] = src[c]
                bcast = sb.tile([P, in_dim], F32, tag=f"{name}_bc")
                for j in range(incols):
                    pT = ps_hd.tile([1, P], F32, tag="hd_tr")
                    nc.tensor.transpose(out=pT, in_=src_col[:, j:j + 1],
                                        identity=ident)
                    rowj = sbw.tile([1, P], F32, tag="hd_row")
                    nc.scalar.activation(out=rowj, in_=pT, func=AF.Copy)
                    pb = ps_hd.tile([P, P], F32, tag="hd_bc")
                    nc.tensor.matmul(out=pb, lhsT=ones_row, rhs=rowj,
                                     start=True, stop=True)
                    nc.scalar.activation(out=bcast[:, j * P:(j + 1) * P], in_=pb,
                                         func=AF.Copy)
                dst = sb.tile([P, och], F32, tag=f"{name}_out")
                for ot in range(och):
                    orw = min(P, out_dim - ot * P)
                    wsb = sbw.tile([P, in_dim], F32, tag=f"{name}_w")
                    nc.sync.dma_start(out=wsb[0:orw, :],
                                      in_=w_dram[ot * P:ot * P + orw, :])
                    prod = sbw.tile([P, in_dim], F32, tag=f"{name}_prod")
                    nc.vector.tensor_mul(prod[0:orw, :], wsb[0:orw, :], bcast[0:orw, :])
                    nc.vector.tensor_reduce(out=dst[0:orw, ot:ot + 1],
                                            in_=prod[0:orw, :], axis=AX.X, op=ALU.add)
                if alpha is not None:
                    nc.vector.scalar_tensor_tensor(
                        out=dst[0:orows, :], in0=dst[0:orows, :], scalar=alpha,
                        in1=dst[0:orows, :], op0=ALU.mult, op1=ALU.max)
                return dst

            y1 = lin("y1", p_cf, 16, t_w["L1"], 512, LRELU_HEAD)
            y2 = lin("y2", y1, 4, t_w["L2"], 256, LRELU_HEAD)
            y3 = lin("y3", y2, 2, t_w["L3"], 128, LRELU_HEAD)
            y4 = lin("y4", y3, 1, t_w["L4"], 2, None)
            osb = sb.tile([2, 1], F32, tag="out_sb")
            nc.vector.tensor_copy(out=osb, in_=y4[0:2, 0:1])
            nc.sync.dma_start(out=t_out["out"][:, :], in_=osb)


_PROG_CACHE = {}


def _build(dbg=False):
    key = ("v3", dbg)
    if key in _PROG_CACHE:
        return _PROG_CACHE[key]
    nc = bacc.Bacc("TRN2", target_bir_lowering=False, debug=False, num_devices=B,
                   num_swdge_queues=NQ)
    t_in = {"feat_xyz": nc.declare_dram_parameter("feat_xyz", [N, 3], F32, isOutput=False)}
    t_w = {}
    for li, (O, C) in enumerate(CONV + [(1024, 512)]):
        wshape = [O, 2 * C] if li < 4 else [O, C]
        t_w[f"W{li}"] = nc.declare_dram_parameter(f"W{li}", wshape, F32, isOutput=False)
        for nm in "gbmv":
            t_w[f"{nm}{li}"] = nc.declare_dram_parameter(f"{nm}{li}", [O, 1], F32,
                                                         isOutput=False)
    for j, (o, c) in enumerate(LIN):
        t_w[f"L{j+1}"] = nc.declare_dram_parameter(f"L{j+1}", [o, c], F32, isOutput=False)
    for li, (O, C) in enumerate(CONV):
        t_w[f"Utab{li}"] = nc.dram_tensor(f"Utab{li}", [N, O],
                                          FP16 if U_FP16[li] else F32)
    t_out = {"out": nc.declare_dram_parameter("out", [2, 1], F32, isOutput=True)}
    if dbg:
        for li, (O, C) in enumerate(CONV):
            sh = [P, 2 * N] if O == 256 else [O, N]
            t_out[f"dbg_x{li}"] = nc.declare_dram_parameter(f"dbg_x{li}", sh, F32,
                                                            isOutput=True)
        t_out["dbg_p"] = nc.declare_dram_parameter("dbg_p", [P, 16], F32, isOutput=True)

    with tile.TileContext(nc) as tc:
        _emit(nc, tc, t_in, t_w, t_out, dbg)
    nc.compile()
    _PROG_CACHE[key] = nc
    return nc


def _make_in_maps(inputs):
    feat = np.ascontiguousarray(np.asarray(inputs["feat_xyz"], dtype=np.float32))
    common = {}
    for li in range(5):
        common[f"W{li}"] = np.ascontiguousarray(np.asarray(inputs[f"W{li}"], np.float32))
        for nm in "gbmv":
            common[f"{nm}{li}"] = np.ascontiguousarray(
                np.asarray(inputs[f"{nm}{li}"], np.float32).reshape(-1, 1))
    for j in range(1, 5):
        common[f"L{j}"] = np.ascontiguousarray(np.asarray(inputs[f"L{j}"], np.float32))
    return [dict(common, feat_xyz=np.ascontiguousarray(feat[b])) for b in range(B)]


def run(inputs, dbg=False, trace=False, **kw):
    nc = _build(dbg)
    in_maps = _make_in_maps(inputs)
    return run_bass_kernel_spmd(nc, in_maps, list(range(B)), trace=trace, **kw)


def kernel(**inputs):
    res = run(inputs).results
    out = np.stack([res[b]["out"][:, 0] for b in range(B)], axis=0)
    return out.astype(np.float32)
